# revision 9
# baseline (speedup 1.0000x reference)
"""Trainium2 Bass kernel for CRPExpertAggregator (moe_routing).

Full-input contract: kernel(**inputs) takes the full unsharded inputs and
returns the full (256, 100) logits. Internally shards batch 8 ways across
NeuronCores 0-7 (data parallel; expert params replicated) and runs one SPMD
Bass program via concourse.bass_utils.run_bass_kernel_spmd.

Math (identical to the reference up to fp reassociation):
  H = x.reshape(B, 64, 256)
  scores[b,el,s] = sum_a (q@Wk/16)[el,a] * H[b,s,a]         (K never formed)
  attn = softmax_s(scores);  attn_avg[bs,e] = 0.25*sum_l attn
  U[b,e,a] = sum_s attn_avg * H;  z[b,e,d] = sum_a U * WvT  (V never formed)
  raw = ||z||, allsc = raw * log(counts+2), top-3 gate, logits = final @ cqT

Precision: the scores path streams fp16 (137x per-row safety on the top-3
gate). The U path needs fp32-level accuracy but fp32 stationaries cost a
213ns LDWEIGHTS per matmul, so attn/attn_avg/H stream as fp16 hi+lo PAIRS
(hi = fp16(x), lo = fp16(x - hi); products accumulate in fp32 PSUM; the
dropped lo*lo term is ~2^-22). The z path stays fully fp32 (rounding Wv or
U to single fp16 leaves <1.6x margin against the smallest rank-3/4 gap).

Perf structure (vs the 56.9us v1):
  - U/pav stationaries fp16 (1-pass LDWEIGHTS instead of 2-pass fp32);
    U streams the avt hi|lo pair concatenated so each stationary load
    serves two products (2 LW + 2 MM per chunk instead of 4 LW + 4 MM)
  - pft/plog run fp16 (z explicitly cast on PSUM evac; final rounding only)
  - raw = rawsq^0.5 on the DVE (AluOpType.pow) so the scalar engine keeps
    the Exp table loaded from scores to gate (saves 2x 1.28us table loads)
  - rw2 gate copies issue per-t inside the z section
"""

import numpy as np

import concourse.bass as bass
import concourse.bacc as bacc
import concourse.mybir as mybir
import concourse.tile as tile
from concourse.bass_utils import run_bass_kernel_spmd
from concourse.alu_op_type import AluOpType

FP32 = mybir.dt.float32
FP16 = mybir.dt.float16
AF = mybir.ActivationFunctionType
AX = mybir.AxisListType

N_CORES = 8
B = 256            # full batch
BL = B // N_CORES  # 32 rows per core
S = 64             # slots
A = 256            # agent dim (contraction for projections)
D = 256            # embed dim
E = 16             # experts
L = 4              # queries per expert
C = 100            # classes
R = BL * S         # 2048 H-rows per core
P = 128

C16W = 2 * S + 2 * C + E + BL  # qwt | cqt | selp | s4


def _build_program():
    nc = bacc.Bacc("TRN2", debug=False, enable_asserts=False, num_devices=N_CORES)

    # Host-packed DRAM inputs (exact SBUF layouts, partition dim first).
    xT = nc.dram_tensor("xT", (P, 2, R), FP16, kind="ExternalInput").ap()
    xnp = nc.dram_tensor("xnp", (P, R // P, 2, A), FP16, kind="ExternalInput").ap()
    wv = nc.dram_tensor("wv", (P, E, 2, D), FP32, kind="ExternalInput").ap()
    c16 = nc.dram_tensor("c16", (P, C16W), FP16, kind="ExternalInput").ap()
    c32 = nc.dram_tensor("c32", (P, E), FP32, kind="ExternalInput").ap()
    out = nc.dram_tensor("out", (BL, C), FP32, kind="ExternalOutput").ap()

    with tile.TileContext(nc) as tc:
        with tc.tile_pool(name="sb", bufs=1) as sb, \
             tc.tile_pool(name="ps", bufs=1, space="PSUM") as ps:
            # ---------------- DMA inputs (order = priority) ----------------
            c16_sb = sb.tile([P, C16W], FP16)
            nc.sync.dma_start(c16_sb, c16)
            xt_sb = sb.tile([P, 2, R], FP16)
            for ac in range(2):
                nc.sync.dma_start(xt_sb[:, ac], xT[:, ac])
            c32_sb = sb.tile([P, E], FP32)
            nc.sync.dma_start(c32_sb, c32)
            xn_sb = sb.tile([P, R // P, 2, A], FP16)  # [bs_p, rc, hi/lo, a]
            for h in range(2):
                nc.sync.dma_start(xn_sb[:, 8 * h:8 * (h + 1)], xnp[:, 8 * h:8 * (h + 1)])
            wv_sb = sb.tile([P, E, 2, D], FP32)
            for g in range(4):
                nc.sync.dma_start(wv_sb[:, 4 * g:4 * (g + 1)], wv[:, 4 * g:4 * (g + 1)])

            qwt = c16_sb[:, 0:2 * S].rearrange("p (ac el) -> p ac el", ac=2)
            cqt = c16_sb[:, 2 * S:2 * S + 2 * C].rearrange("p (dc c) -> p dc c", dc=2)
            selp = c16_sb[:, 2 * S + 2 * C:2 * S + 2 * C + E]
            s4 = c16_sb[:, 2 * S + 2 * C + E:]
            crp = c32_sb[0:BL, :]

            # Prefetch the Exp activation table while DMAs stream.
            warm_in = sb.tile([1, 1], FP32)
            warm_out = sb.tile([1, 2], FP32)
            nc.vector.memset(warm_in, 0.0)
            nc.scalar.activation(warm_out[:, 0:1], warm_in, AF.Exp)

            # ------- scores (fp16 mm, 2-way col tiling) -> exp -> normalize ----
            # attn layout [p = 64*h + el, bb = b%16, s]; h = b//16.
            attn = sb.tile([P, E, S], FP32)
            den = sb.tile([P, E], FP32)
            rden = sb.tile([P, E], FP32)
            ah = sb.tile([P, E, S], FP16)   # fp16 hi of normalized attn
            al = sb.tile([P, E, S], FP16)   # fp16 lo residual
            for it in range(2):
                psc = ps.tile([P, 8, S], FP32, tag="sc", bufs=2)
                for h in range(2):
                    for ac in range(2):
                        nc.tensor.matmul(
                            psc[64 * h:64 * (h + 1)].rearrange("p b s -> p (b s)"),
                            qwt[:, ac, :],
                            xt_sb[:, ac, 1024 * h + 512 * it:1024 * h + 512 * (it + 1)],
                            start=(ac == 0), stop=(ac == 1),
                            tile_position=(0, 64 * h),
                        )
                sl = slice(8 * it, 8 * (it + 1))
                nc.scalar.activation(attn[:, sl, :], psc, AF.Exp)
                nc.vector.reduce_sum(den[:, sl], attn[:, sl, :], axis=AX.X)
                nc.vector.reciprocal(rden[:, sl], den[:, sl])
                nc.vector.tensor_tensor(
                    attn[:, sl, :], attn[:, sl, :],
                    rden[:, sl, None].to_broadcast((P, 8, S)),
                    AluOpType.mult,
                )
                nc.scalar.copy(ah[:, sl, :], attn[:, sl, :])
                nc.gpsimd.tensor_tensor(
                    al[:, sl, :], attn[:, sl, :], ah[:, sl, :], AluOpType.subtract)

            # ------- attn_avg^T (2-way row tiling, fp16 hi/lo stationaries) ----
            # avtp[r, rc, pair, par, e]: pair 0 = fp16 hi of attn_avg, pair 1 =
            # fp16 lo residual; parity par as before (complement rows zero).
            avtp = sb.tile([P, R // P, 2, 2, E], FP16)
            nc.vector.memset(avtp[S:P, :, :, 0, :], 0.0)
            nc.vector.memset(avtp[:S, :, :, 1, :], 0.0)
            pav0 = ps.tile([P, 8, E], FP32, tag="gp", bufs=3)
            pav1 = ps.tile([P, 8, E], FP32, tag="gp", bufs=3)
            pav = [pav0, pav1]
            for it in range(2):
                for k in range(4):
                    pl = 4 * it + k
                    for h in range(2):
                        for pr, src in ((0, ah), (1, al)):
                            nc.tensor.matmul(
                                pav[h][:, pl, :],
                                src[64 * h:64 * (h + 1), 2 * pl:2 * pl + 2, :]
                                .rearrange("p b s -> p (b s)"),
                                selp[64 * h:64 * (h + 1), :],
                                start=(pr == 0), stop=(pr == 1),
                                tile_position=(64 * h, 0),
                            )
            for h in range(2):
                hs = slice(8 * h, 8 * (h + 1))
                for par, rs in ((0, slice(0, S)), (1, slice(S, P))):
                    nc.vector.tensor_copy(avtp[rs, hs, 0, par, :], pav[h][rs])
                    nc.vector.tensor_tensor(
                        avtp[rs, hs, 1, par, :], pav[h][rs],
                        avtp[rs, hs, 0, par, :], AluOpType.subtract)

            # ------- U^T [a, b, e] = sum_s H^T attn_avg (fp16 pairs -> fp32) ---
            # Per chunk: Hh @ [avt_hi | avt_lo] (one LW, free=64), then
            # Hl @ avt_hi accumulating into the hi half (one LW, free=32).
            # ut = hi-half + lo-half of the psum afterwards.
            ut = sb.tile([P, 2, E, BL], FP32)  # [a_p, a_c, e, b]
            for ac in range(2):
                for half in range(2):
                    pu = ps.tile([P, 8, 2, 2, E], FP32, tag="gp", bufs=3)
                    for i in range(8):
                        rc = 8 * half + i
                        nc.tensor.matmul(
                            pu[:, i].rearrange("p pr par e -> p (pr par e)"),
                            xn_sb[:, rc, 0, 128 * ac:128 * (ac + 1)],
                            avtp[:, rc].rearrange("p pr par e -> p (pr par e)"),
                            start=True, stop=False,
                        )
                        nc.tensor.matmul(
                            pu[:, i, 0].rearrange("p par e -> p (par e)"),
                            xn_sb[:, rc, 1, 128 * ac:128 * (ac + 1)],
                            avtp[:, rc, 0].rearrange("p par e -> p (par e)"),
                            start=False, stop=True,
                            skip_group_check=True,
                        )
                    ut_view = ut[:, ac, :, 16 * half:16 * (half + 1)] \
                        .rearrange("p e (i par) -> p i par e", par=2)
                    nc.scalar.copy(ut_view, pu[:, :, 0])
                    nc.vector.tensor_tensor(ut_view, pu[:, :, 1], ut_view,
                                            AluOpType.add)

            # Prefetch the Sqrt table under the z section (dep on ut forces
            # this to execute after U, not at program start).
            nc.scalar.sqrt(warm_out[:, 1:2], ut[0:1, 0, 0, 0:1])

            # ------- z [32j+b, t, d], expert e = 4t+j (fp32, 4-way col tiling) -
            z_sb = sb.tile([P, 4, D], FP16)   # final-path copy (pft is fp16)
            zsq = sb.tile([P, 4, D], FP32)
            rawsq = sb.tile([P, 4], FP32)
            raw = sb.tile([P, 4], FP32)
            rw2 = sb.tile([BL, 4, 4], FP32)   # [b, t, j] -> free index e=4t+j
            for t in range(4):
                pz = ps.tile([P, D], FP32, tag="z", bufs=3)
                for j in range(4):
                    e = 4 * t + j
                    for ac in range(2):
                        nc.tensor.matmul(
                            pz[32 * j:32 * (j + 1), :],
                            ut[:, ac, e, :],
                            wv_sb[:, e, ac, :],
                            start=(ac == 0), stop=(ac == 1),
                            tile_position=(0, 32 * j),
                        )
                nc.scalar.activation(zsq[:, t, :], pz, AF.Square,
                                     accum_out=rawsq[:, t:t + 1])
                nc.vector.tensor_copy(z_sb[:, t, :], pz)
                nc.scalar.sqrt(raw[:, t:t + 1], rawsq[:, t:t + 1])
                for j in range(4):
                    nc.vector.tensor_copy(rw2[:, t:t + 1, j],
                                          raw[32 * j:32 * (j + 1), t:t + 1])

            # Warm the Exp table during the z tail (dep on the last raw column).
            nc.scalar.activation(warm_out[:, 0:1], raw[0:1, 3:4], AF.Exp)

            # ---------------- allsc = raw * crp, top-3 gate --------------------
            allsc = sb.tile([BL, E], FP32)
            nc.vector.tensor_tensor(
                allsc.rearrange("p (t j) -> p t j", j=4), rw2,
                crp.rearrange("p (t j) -> p t j", j=4), AluOpType.mult)
            mx8 = sb.tile([BL, 8], FP32)
            nc.vector.max(mx8, allsc)
            negm1 = sb.tile([BL, 1], FP32)
            nc.vector.tensor_scalar_mul(negm1, mx8[:, 0:1], -1.0)
            g = sb.tile([BL, E], FP32)
            nc.scalar.activation(g, allsc, AF.Exp, bias=negm1)
            gm = sb.tile([BL, E], FP32)
            nc.vector.scalar_tensor_tensor(
                gm, allsc, mx8[:, 2:3], g, AluOpType.is_ge, AluOpType.mult)
            ssum = sb.tile([BL, 1], FP32)
            nc.vector.reduce_sum(ssum, gm, axis=AX.X)
            rsum = sb.tile([BL, 1], FP32)
            nc.vector.reciprocal(rsum, ssum)
            we = sb.tile([BL, E], FP32)
            nc.vector.tensor_scalar_mul(we, gm, rsum)

            # scatter we [b, e] -> we128 [32j+b, t]; wsel[p, t, b] = s4 * we128
            we128 = sb.tile([P, 4], FP32)
            wev = we.rearrange("p (t j) -> p t j", j=4)
            for j in range(4):
                nc.vector.tensor_copy(we128[32 * j:32 * (j + 1), :], wev[:, :, j])
            wsel = sb.tile([P, 4, BL], FP16)
            for t in range(4):
                nc.vector.tensor_scalar_mul(wsel[:, t, :], s4, we128[:, t:t + 1])

            # final^T [d, b] = sum_{p,t} z[p, t, d] * wsel[p, t, b]   (fp16 mm)
            pft = ps.tile([P, 2, BL], FP32, tag="gp", bufs=3)
            for dc in range(2):
                for t in range(4):
                    nc.tensor.matmul(
                        pft[:, dc, :],
                        z_sb[:, t, 128 * dc:128 * (dc + 1)],
                        wsel[:, t, :],
                        start=(t == 0), stop=(t == 3),
                    )
            ft16 = sb.tile([P, 2, BL], FP16)
            nc.vector.tensor_copy(ft16, pft)

            # logits [b, c] = sum_d final^T[d, b] * cq^T[d, c]   (fp16 mm)
            plog = ps.tile([BL, C], FP32, tag="gp", bufs=3)
            for dc in range(2):
                nc.tensor.matmul(
                    plog, ft16[:, dc, :], cqt[:, dc, :],
                    start=(dc == 0), stop=(dc == 1),
                )
            out_sb = sb.tile([BL, C], FP32)
            nc.vector.tensor_copy(out_sb, plog)
            nc.sync.dma_start(out, out_sb)

    nc.compile()
    # compile()'s move_matmul_waits_to_ldweights runs before the final ISA
    # lowering splits fused matmuls into Ldweights+Matmult, so a matmul can
    # still carry 2 waits (walrus MM struct fits only 1). Re-run the passes.
    import bass_rust
    bass_rust.move_matmul_waits_to_ldweights(nc.m)
    bass_rust.generate_event_semaphores(nc)
    for f in nc.m.functions:
        for blk in f.blocks:
            for inst in blk.instructions:
                w = inst.sync_info.on_wait if inst.sync_info else None
                if w and len(w) > 1 and "EventSemaphore" not in str(inst.opcode):
                    raise RuntimeError(
                        f"{inst.name} {inst.opcode} still has {len(w)} waits")
    return nc


_NC = None


def _get_nc():
    global _NC
    if _NC is None:
        _NC = _build_program()
    return _NC


def _make_in_maps(inputs):
    x = np.ascontiguousarray(np.asarray(inputs["x"], dtype=np.float32))
    queries = np.asarray(inputs["queries"], dtype=np.float64)
    Wk = np.asarray(inputs["Wk"], dtype=np.float64)
    Wv = np.asarray(inputs["Wv"], dtype=np.float32)
    cq = np.asarray(inputs["class_queries"], dtype=np.float32)
    counts = np.asarray(inputs["expert_counts"]).astype(np.float64)

    # c16 [128, C16W] fp16: qwt | cqt | selp | s4
    qw = (np.einsum("eld,eda->ela", queries, Wk) / 16.0).astype(np.float32)
    qwT = qw.reshape(E * L, A).T.reshape(2, P, E * L).transpose(1, 0, 2)
    cqT = cq.T.reshape(2, P, C).transpose(1, 0, 2)
    selp = np.zeros((P, E), np.float32)
    s4 = np.zeros((P, BL), np.float32)
    for p in range(P):
        selp[p, (p % S) // L] = 0.25
        s4[p, p % BL] = 1.0
    c16 = np.concatenate(
        [qwT.reshape(P, 2 * S), cqT.reshape(P, 2 * C), selp, s4],
        axis=1).astype(np.float16)
    c16 = np.ascontiguousarray(c16)

    c32 = np.zeros((P, E), np.float32)
    c32[0:BL, :] = np.log(counts + 2.0).astype(np.float32).reshape(1, E)

    # wv [128, e, ac, d] fp32
    wvp = np.ascontiguousarray(
        Wv.transpose(0, 2, 1).reshape(E, 2, P, D).transpose(2, 0, 1, 3))

    in_maps = []
    for c in range(N_CORES):
        xl = x[BL * c:BL * (c + 1)].reshape(R, A)
        xTp = np.ascontiguousarray(
            xl.T.reshape(2, P, R).transpose(1, 0, 2)).astype(np.float16)
        # xnp [128, rc, hi/lo, a] fp16 pair of the natural layout
        xn = xl.reshape(R // P, P, A).transpose(1, 0, 2)
        hi = xn.astype(np.float16)
        lo = (xn - hi.astype(np.float32)).astype(np.float16)
        xnpair = np.ascontiguousarray(np.stack([hi, lo], axis=2))
        in_maps.append({
            "xT": xTp,
            "xnp": xnpair,
            "wv": wvp,
            "c16": c16,
            "c32": c32,
        })
    return in_maps


def run_sharded(inputs, trace=False, **kwargs):
    nc = _get_nc()
    in_maps = _make_in_maps(inputs)
    res = run_bass_kernel_spmd(nc, in_maps, core_ids=list(range(N_CORES)),
                               trace=trace, **kwargs)
    outs = np.concatenate([res.results[c]["out"] for c in range(N_CORES)], axis=0)
    return outs.astype(np.float32), res


def kernel(**inputs):
    out, _ = run_sharded(inputs, trace=False)
    return out
